# revision 48
# baseline (speedup 1.0000x reference)
"""ApproxNDCG loss kernel for Trainium2, distributed over 8 NeuronCores.

Strategy (data-parallel over batch dim B=32; 4 rows per core):

For each row (list of L=2048 items):
  soft_ranks_i  = 1 + sum_{j != i} sigmoid(p_i - p_j)
  hard_rank0_i  = #{j : t_j > t_i}            (position in descending sort)
  gains_i       = 2^t_i - 1
  approx_dcg    = sum_i gains_i / log2(1 + soft_ranks_i)
  ideal_dcg     = sum_i gains_i / log2(2 + hard_rank0_i)
  loss          = mean_rows(1 - approx_dcg / ideal_dcg)

(the hard-rank formulation of ideal_dcg is exact up to float ties, whose
contribution is invariant because tied targets have equal gains).

Both pairwise matrices are only computed on their upper triangle
(row-chunk trapezoids, j >= 128*I), in bf16, with the fused per-partition
accum_out giving the direct row sums.  The lower triangle is
reconstructed from (anti)symmetry with TensorEngine column reductions of
the already-computed trapezoids (bf16 weights -> fast LDWEIGHTS):
  sigma(p_i - p_j) = 1 - sigma(p_j - p_i)
  (t_j > t_i)      = 1 - (t_i > t_j)   (up to measure-zero ties)
ScalarE computes sigmoid trapezoids; VectorE computes is_gt trapezoids;
TensorE folds both mirror terms into one PSUM bank per row.
"""

import math
from contextlib import ExitStack

import numpy as np

import concourse.bass as bass
import concourse.tile as tile
from concourse import bacc, mybir
from concourse.bass_utils import run_bass_kernel_spmd

B, L = 32, 2048
NCORES = 8
ROWS = B // NCORES          # rows of the batch per core
P = 128                     # SBUF partitions
NCH = L // P                # 16 row-chunks per row
F32 = mybir.dt.float32
BF16 = mybir.dt.bfloat16
LN2 = math.log(2.0)

AF = mybir.ActivationFunctionType
OP = mybir.AluOpType


def _emit(ctx: ExitStack, tc: "tile.TileContext", pred: bass.AP, targ: bass.AP,
          out: bass.AP, dbg: dict | None = None) -> None:
    nc = tc.nc

    rows_pool = ctx.enter_context(tc.tile_pool(name="rows", bufs=2))
    rowvec_pool = ctx.enter_context(tc.tile_pool(name="rowvec", bufs=3))
    rep_pool = ctx.enter_context(tc.tile_pool(name="rep", bufs=2))
    trap_pool = ctx.enter_context(tc.tile_pool(name="trap", bufs=6))
    cmp_pool = ctx.enter_context(tc.tile_pool(name="cmp", bufs=6))
    small = ctx.enter_context(tc.tile_pool(name="small", bufs=1))
    psum_mir = ctx.enter_context(
        tc.tile_pool(name="mir", bufs=ROWS, space="PSUM"))
    psum_red = ctx.enter_context(tc.tile_pool(name="red", bufs=1, space="PSUM"))
    psum_tp = ctx.enter_context(tc.tile_pool(name="tp", bufs=2, space="PSUM"))

    # --- constants -----------------------------------------------------
    ones_bf = small.tile([P, 1], BF16, tag="ones_bf")
    nc.vector.memset(ones_bf[:], 1.0)
    ones_col = small.tile([P, 1], F32, tag="ones_col")
    nc.vector.memset(ones_col[:], 1.0)
    two_col = small.tile([P, 1], F32, tag="two_col")
    nc.vector.memset(two_col[:], 2.0)
    # identity via iota (standard gpsimd lib; avoids the affine_select
    # library switch) + DVE compare
    it_i = small.tile([NCH, NCH], mybir.dt.int32, tag="it_i")
    nc.gpsimd.iota(it_i[:], pattern=[[-1, NCH]], base=0, channel_multiplier=1)
    ident = small.tile([NCH, NCH], F32, tag="ident")
    nc.vector.tensor_scalar(ident[:], it_i[:], 0, None, op0=OP.is_equal)
    # per-chunk offsets: soft Ln arg 128 I + 1.5, ideal Ln arg 128 I + 2
    const_soft = small.tile([P, NCH], F32, tag="const_soft")
    const_ideal = small.tile([P, NCH], F32, tag="const_ideal")
    for I in range(NCH):
        nc.vector.memset(const_soft[:, I:I + 1], 128.0 * I + 1.5)
        nc.vector.memset(const_ideal[:, I:I + 1], 128.0 * I + 2.0)

    # persistent per-row stats, rows side by side in the free dim
    pT_all = small.tile([P, NCH * ROWS], F32, tag="pT_all")
    tT_all = small.tile([P, NCH * ROWS], F32, tag="tT_all")
    sig_all = small.tile([P, NCH * ROWS], F32, tag="sig_all")
    cnt_all = small.tile([P, NCH * ROWS], F32, tag="cnt_all")
    gm1_all = small.tile([P, NCH * ROWS], F32, tag="gm1_all")
    s2_all = small.tile([P, NCH * ROWS], F32, tag="s2_all")
    s4_all = small.tile([P, NCH * ROWS], F32, tag="s4_all")
    # numerator sums in cols [0, ROWS), denominator sums in [ROWS, 2*ROWS)
    acc_all = small.tile([P, 2 * ROWS], F32, tag="acc_all")

    # [16, 128] row views for the PE chunk-transpose: pT[q, f] = p[128 f + q]
    predC = pred.rearrange("b (a c) -> b a c", a=NCH)
    targC = targ.rearrange("b (a c) -> b a c", a=NCH)

    neg1_16 = small.tile([NCH, 1], F32, tag="neg1_16")
    nc.vector.memset(neg1_16[:], -1.0)

    # --- phase A: loads, transposes, gains.  Gains use sigmoid algebra,
    # 2^t - 1 = (2 s - 1) / (1 - s) with s = sigmoid(t ln2), so the whole
    # kernel needs only the sigmoid and natural_log ACT table sets (an Exp
    # would thrash table loads against the sigmoid stream).
    p_rows, t_rows = [], []
    for r in range(ROWS):
        pT = pT_all[:, r * NCH:(r + 1) * NCH]
        tT = tT_all[:, r * NCH:(r + 1) * NCH]
        if r == 0:
            p_rows.append(None)
            t_rows.append(None)
        else:
            p_row = rowvec_pool.tile([1, L], F32, tag="rowvec")
            nc.sync.dma_start(p_row[:], pred[r:r + 1, :])
            t_row = rowvec_pool.tile([1, L], F32, tag="rowvec")
            nc.sync.dma_start(t_row[:], targ[r:r + 1, :])
            p_rows.append(p_row)
            t_rows.append(t_row)
        c16p = rows_pool.tile([NCH, P], F32, tag="c16p")
        nc.sync.dma_start(c16p[:], predC[r])
        c16t = rows_pool.tile([NCH, P], F32, tag="c16t")
        nc.sync.dma_start(c16t[:], targC[r])
        s16 = rows_pool.tile([NCH, P], F32, tag="s16")
        nc.scalar.activation(s16[:], c16t[:], AF.Sigmoid, scale=LN2)
        a16 = rows_pool.tile([NCH, P], F32, tag="a16")
        nc.scalar.activation(a16[:], s16[:], AF.Identity,
                             bias=neg1_16[:], scale=2.0)
        b16 = rows_pool.tile([NCH, P], F32, tag="b16")
        nc.scalar.activation(b16[:], s16[:], AF.Identity,
                             bias=1.0, scale=-1.0)
        rb16 = rows_pool.tile([NCH, P], F32, tag="rb16")
        nc.vector.reciprocal(rb16[:], b16[:])
        g16 = rows_pool.tile([NCH, P], F32, tag="g16")
        nc.vector.tensor_tensor(g16[:], a16[:], rb16[:], op=OP.mult)
        tp_p = psum_tp.tile([P, NCH], F32, tag="tp")
        nc.tensor.transpose(tp_p[:], c16p[:], ident[:])
        nc.vector.tensor_copy(pT, tp_p[:])
        tp_t = psum_tp.tile([P, NCH], F32, tag="tp")
        nc.tensor.transpose(tp_t[:], c16t[:], ident[:])
        nc.vector.tensor_copy(tT, tp_t[:])
        tp_g = psum_tp.tile([P, NCH], F32, tag="tp")
        nc.tensor.transpose(tp_g[:], g16[:], ident[:])
        nc.vector.tensor_copy(gm1_all[:, r * NCH:(r + 1) * NCH], tp_g[:])

    # --- phase B: pairwise trapezoids + mirror column sums -------------
    mirs = []
    for r in range(ROWS):
        pT = pT_all[:, r * NCH:(r + 1) * NCH]
        tT = tT_all[:, r * NCH:(r + 1) * NCH]
        sig_acc = sig_all[:, r * NCH:(r + 1) * NCH]
        cnt_acc = cnt_all[:, r * NCH:(r + 1) * NCH]

        # replicate the row across all partitions.  Row 0 goes over split
        # 0-stride DMAs — they finish inside the idle startup window and
        # don't wait for the gpsimd library load.  Later rows use the
        # otherwise-idle GpSimd (a DMA broadcast during compute costs ~10%
        # on both ACT and DVE streams via SBUF port contention).
        p_rep = rep_pool.tile([P, L], F32, tag="p_rep")
        t_rep = rep_pool.tile([P, L], F32, tag="t_rep")
        if r == 0:
            step = P // 4
            psrc = pred[0:1, :].partition_broadcast(step)
            tsrc = targ[0:1, :].partition_broadcast(step)
            for s in range(4):
                nc.sync.dma_start(p_rep[s * step:(s + 1) * step, :], psrc)
                nc.sync.dma_start(t_rep[s * step:(s + 1) * step, :], tsrc)
        else:
            nc.gpsimd.partition_broadcast(p_rep[:], p_rows[r][:])
            nc.gpsimd.partition_broadcast(t_rep[:], t_rows[r][:])

        # one PSUM bank per row: cols [0,16) sigma-mirror, [16,32) cnt-mirror
        mir = psum_mir.tile([P, 2 * NCH], F32, tag="mir")
        mirs.append(mir)
        nc.vector.memset(mir[:, 0:1], 0.0)
        nc.vector.memset(mir[:, NCH:NCH + 1], 0.0)

        for I in range(NCH):
            W = L - P * I
            # sigma[i, j] = sigmoid(p_i - p_j), i = 128 I + q, j >= 128 I
            trap = trap_pool.tile([P, L], BF16, tag="trap")
            nc.scalar.activation(
                trap[:, :W], p_rep[:, P * I:], AF.Sigmoid,
                bias=pT[:, I:I + 1], scale=-1.0,
                accum_out=sig_acc[:, I:I + 1])
            # cmp[i, j] = (t_j > t_i)
            cmpt = cmp_pool.tile([P, L], BF16, tag="cmp")
            nc.vector.tensor_scalar(
                cmpt[:, :W], t_rep[:, P * I:], tT[:, I:I + 1], None,
                op0=OP.is_gt, op1=OP.add,
                accum_out=cnt_acc[:, I:I + 1])
            # mirror column sums; the whole mir bank is ONE accumulation
            # group (start pending-zeroes the 2KB zero region, so each
            # column's first contribution overwrites and later ones add)
            for I2 in range(I + 1, NCH):
                o = P * (I2 - I)
                nc.tensor.matmul(
                    mir[:, I2:I2 + 1],
                    lhsT=trap[:, o:o + P], rhs=ones_bf[:],
                    start=(I == 0 and I2 == 1), stop=False,
                    skip_group_check=True)
                nc.tensor.matmul(
                    mir[:, NCH + I2:NCH + I2 + 1],
                    lhsT=cmpt[:, o:o + P], rhs=ones_bf[:],
                    start=False,
                    stop=(I == NCH - 2 and I2 == NCH - 1),
                    skip_group_check=True)

        # fold this row's mirrors into the Ln arguments now, while the
        # next row still streams (shrinks the post-stream tail):
        #   soft  arg = sig_acc + (128 I - mir_sig) + 1.5
        #   ideal arg = cnt_acc + (128 I - mir_cnt) + 2
        s1 = small.tile([P, NCH], F32, tag="s1")
        nc.vector.tensor_tensor(s1[:], sig_acc, mir[:, 0:NCH], op=OP.subtract)
        nc.vector.tensor_tensor(s2_all[:, r * NCH:(r + 1) * NCH], s1[:],
                                const_soft[:], op=OP.add)
        s3 = small.tile([P, NCH], F32, tag="s3")
        nc.vector.tensor_tensor(s3[:], cnt_acc, mir[:, NCH:2 * NCH],
                                op=OP.subtract)
        nc.vector.tensor_tensor(s4_all[:, r * NCH:(r + 1) * NCH], s3[:],
                                const_ideal[:], op=OP.add)

    # --- phase C: epilogue (Ln ACTs batched) ---------------------------
    for r in range(ROWS):
        gm1 = gm1_all[:, r * NCH:(r + 1) * NCH]
        ln_s = small.tile([P, NCH], F32, tag="ln_s")
        nc.scalar.activation(ln_s[:], s2_all[:, r * NCH:(r + 1) * NCH], AF.Ln)
        ln_i = small.tile([P, NCH], F32, tag="ln_i")
        nc.scalar.activation(ln_i[:], s4_all[:, r * NCH:(r + 1) * NCH], AF.Ln)

        inv_s = small.tile([P, NCH], F32, tag="inv_s")
        nc.vector.reciprocal(inv_s[:], ln_s[:])
        inv_i = small.tile([P, NCH], F32, tag="inv_i")
        nc.vector.reciprocal(inv_i[:], ln_i[:])
        prod_a = small.tile([P, NCH], F32, tag="prod_a")
        nc.vector.tensor_tensor(prod_a[:], gm1, inv_s[:], op=OP.mult)
        nc.vector.reduce_sum(acc_all[:, r:r + 1], prod_a[:],
                             axis=mybir.AxisListType.X)
        prod_b = small.tile([P, NCH], F32, tag="prod_b")
        nc.vector.tensor_tensor(prod_b[:], gm1, inv_i[:], op=OP.mult)
        nc.vector.reduce_sum(acc_all[:, ROWS + r:ROWS + r + 1], prod_b[:],
                             axis=mybir.AxisListType.X)

    if dbg is not None:
        nc.sync.dma_start(dbg["sig"][:, :], sig_all[:])
        nc.sync.dma_start(dbg["cnt"][:, :], cnt_all[:])
        for r in range(ROWS):
            mcopy = small.tile([P, 2 * NCH], F32, tag=f"mcopy{r}")
            nc.vector.tensor_copy(mcopy[:], mirs[r][:])
            nc.sync.dma_start(dbg["mir"][:, r * 2 * NCH:(r + 1) * 2 * NCH],
                              mcopy[:])
        nc.sync.dma_start(dbg["acc"][:, :], acc_all[:])

    # partition-reduce the per-partition partial sums: [128, R] -> [R, 1]
    # (lhsT = acc columns so M = ROWS; an M=1 ones-lhsT matmul is avoided)
    num_red = psum_red.tile([ROWS, 1], F32, tag="num_red")
    nc.tensor.matmul(num_red[:], lhsT=acc_all[:, 0:ROWS], rhs=ones_col[:],
                     start=True, stop=True)
    den_red = psum_red.tile([ROWS, 1], F32, tag="den_red")
    nc.tensor.matmul(den_red[:], lhsT=acc_all[:, ROWS:2 * ROWS],
                     rhs=ones_col[:], start=True, stop=True)

    num_sb = small.tile([ROWS, 1], F32, tag="num_sb")
    nc.vector.tensor_copy(num_sb[:], num_red[:])
    den_sb = small.tile([ROWS, 1], F32, tag="den_sb")
    nc.vector.tensor_copy(den_sb[:], den_red[:])
    inv_den = small.tile([ROWS, 1], F32, tag="inv_den")
    nc.vector.reciprocal(inv_den[:], den_sb[:])
    ratio = small.tile([ROWS, 1], F32, tag="ratio")
    nc.vector.tensor_tensor(ratio[:], num_sb[:], inv_den[:], op=OP.mult)
    rowloss = small.tile([ROWS, 1], F32, tag="rowloss")
    nc.vector.tensor_scalar(rowloss[:], ratio[:], -1.0, 1.0,
                            op0=OP.mult, op1=OP.add)
    nc.sync.dma_start(out[:, :], rowloss[:])


def build(debug: bool = False) -> bass.Bass:
    nc = bacc.Bacc(trn_type="TRN2")
    pred = nc.dram_tensor("predictions", [ROWS, L], F32, kind="ExternalInput")
    targ = nc.dram_tensor("targets", [ROWS, L], F32, kind="ExternalInput")
    out = nc.dram_tensor("out", [ROWS, 1], F32, kind="ExternalOutput")
    dbg = None
    if debug:
        dbg = {
            "sig": nc.dram_tensor("dbg_sig", [P, NCH * ROWS], F32,
                                  kind="ExternalOutput").ap(),
            "cnt": nc.dram_tensor("dbg_cnt", [P, NCH * ROWS], F32,
                                  kind="ExternalOutput").ap(),
            "mir": nc.dram_tensor("dbg_mir", [P, 2 * NCH * ROWS], F32,
                                  kind="ExternalOutput").ap(),
            "acc": nc.dram_tensor("dbg_acc", [P, 2 * ROWS], F32,
                                  kind="ExternalOutput").ap(),
        }
    with tile.TileContext(nc) as tc:
        with ExitStack() as ctx:
            _emit(ctx, tc, pred.ap(), targ.ap(), out.ap(), dbg)
    nc.compile()
    return nc


def make_in_maps(predictions: np.ndarray, targets: np.ndarray):
    predictions = np.ascontiguousarray(predictions, dtype=np.float32)
    targets = np.ascontiguousarray(targets, dtype=np.float32)
    return [
        {
            "predictions": predictions[c * ROWS:(c + 1) * ROWS],
            "targets": targets[c * ROWS:(c + 1) * ROWS],
        }
        for c in range(NCORES)
    ]


def kernel(predictions: np.ndarray, targets: np.ndarray, _trace: bool = False,
           **_run_kwargs):
    nc = build()
    in_maps = make_in_maps(predictions, targets)
    res = run_bass_kernel_spmd(nc, in_maps, core_ids=list(range(NCORES)),
                               trace=_trace, **_run_kwargs)
    partial = sum(float(r["out"][:, 0].sum()) for r in res.results)
    loss = np.float32(partial / B)
    if _trace:
        return np.asarray(loss), res
    return np.asarray(loss)
